# revision 20
# baseline (speedup 1.0000x reference)
"""Trainium2 Bass kernel for nn_BasicLSTM (B=64, T=512, D=512, U=1024).

Data-parallel over batch across 8 NeuronCores (8 sequences per core, the
recurrence fully local per core — no cross-core communication).

Per-core schedule, per step t:
  z(t) = [x_t, h, 1] @ W accumulates in two PSUM banks (units 0:512 and
  512:1024), each holding the four gates (i, f, g, o) at partition
  offsets (0, 32, 64, 96) via 4-way PE column-group packing (M=8).
  The x/bias part runs two steps ahead (it fills the PE during the
  cell-update tail and keeps the HAM clock-gate from re-throttling);
  the h part uses hT produced by the previous step's tail.

  The cell update runs in unit-major layout: sigmoid(z) (one ACT pass
  per bank; g-gate columns host-prescaled by 2 so tanh(x) =
  2*sigmoid(2x)-1 needs no second ACT pass) is PE-transposed into
  sT[128 units, gate*32 + chunk*8 + batch] so every DVE tail op is a
  [128, 32] tensor (all 128 lanes busy) instead of [8, 512] (8 lanes).
  The tail's hT output IS the lhsT layout the next h-matmul needs.

  HW quirk: in this instruction mix, PE transposes pinned to row-groups
  other than 0 abort the NEFF, so the f/g/o gate rows (partitions
  32/64/96) are DVE-moved down to partition 0 (fgo_sb) first; the moves
  are pipelined with the transposes at per-gate granularity.

Matmul operands are bf16 (fp32 PSUM accumulation); c stays fp32.
"""

import numpy as np
import ml_dtypes

B, T, D, U = 64, 512, 512, 1024
NCORES = 8
BL = B // NCORES          # 8 sequences per core
NKX = D // 128            # 4 x K-chunks
NKH = U // 128            # 8 h K-chunks
NT = 512                  # N-tile width (one PSUM bank)
GOFF = (0, 32, 64, 96)    # PSUM partition offset per col-group (i,f,g,o)


def _build_nc(t_steps=T):
    import concourse.bass as bass
    import concourse.mybir as mybir

    f32, bf16 = mybir.dt.float32, mybir.dt.bfloat16
    AF = mybir.ActivationFunctionType
    ALU = mybir.AluOpType

    TB = t_steps * BL

    nc = bass.Bass(num_devices=NCORES)
    wq = nc.declare_dram_parameter("wq", [1537, 4096], bf16, isOutput=False)
    xq = nc.declare_dram_parameter("xq", [NKX, 128, t_steps, BL], bf16, isOutput=False)
    idb_d = nc.declare_dram_parameter("idb", [9, 128], bf16, isOutput=False)
    idr_d = nc.declare_dram_parameter("idr", [104, 8], bf16, isOutput=False)
    out_d = nc.declare_dram_parameter("out", [128, 64], f32, isOutput=True)

    from contextlib import ExitStack
    ctx = ExitStack()
    sb = lambda shape, dt, name: ctx.enter_context(nc.sbuf_tensor(name, shape, dt))
    ps = lambda shape, dt, name: ctx.enter_context(nc.psum_tensor(name, shape, dt))
    sem = lambda name: ctx.enter_context(nc.semaphore(name))

    with ctx:
        w_sb = sb([128, (NKX + NKH) * 4096], bf16, "w_sb")   # Wx then Wh chunks
        bias_sb = sb([1, 4096], bf16, "bias_sb")
        x_sb = sb([128, NKX * TB], bf16, "x_sb")
        s_sb = [[sb([128, NT], bf16, f"s{d}{k}") for k in range(2)] for d in range(2)]
        ones_sb = sb([1, 128], bf16, "ones_sb")
        idr_sb = sb([104, 8], bf16, "idr_sb")
        fgo_sb = [sb([32, 3 * NT], bf16, f"fgo{i}") for i in range(2)]
        gm_sb = sb([128, 64], bf16, "gm_sb")
        t1_sb = sb([128, 64], bf16, "t1_sb")
        c1_sb = sb([128, 64], f32, "c1_sb")
        cT = sb([128, 64], f32, "cT")
        tc_sb = sb([128, 64], bf16, "tc_sb")
        hT = [sb([128, 64], bf16, f"hT{i}") for i in range(2)]
        hlastT = sb([128, 64], f32, "hlastT")

        zb = [ps([128, NT], f32, f"zb{i}") for i in range(4)]
        # full-bank sized so no two share a physical PSUM bank (PE writes one
        # while DVE reads another; same-bank overlap is fatal on HW)
        sT = [[ps([128, 1024], bf16, f"sT{d}{k}") for k in range(2)] for d in range(2)]

        dma_sem = sem("dma_sem")
        mm_sem = sem("mm_sem")
        sig_sem = sem("sig_sem")
        oc_sem = sem("oc_sem")     # 6 incs/step: (fA,gA,oA,fB,gB,oB)
        tr_sem = sem("tr_sem")
        c_sem = sem("c_sem")
        tanh_sem = sem("tanh_sem")
        h_sem = sem("h_sem")

        N_DMAS = (NKX + NKH) + NKX + 3   # w, x, idb, idr, bias

        with nc.Block() as block:

            @block.sync
            def _(sync):
                for kc in range(NKX + NKH):
                    sync.dma_start(
                        out=w_sb[:, kc * 4096:(kc + 1) * 4096],
                        in_=wq[kc * 128:(kc + 1) * 128, :],
                    ).then_inc(dma_sem, 16)
                for kc in range(NKX):
                    sync.dma_start(
                        out=x_sb[:, kc * TB:(kc + 1) * TB],
                        in_=xq[kc],
                    ).then_inc(dma_sem, 16)
                sync.dma_start(out=ones_sb[:, :], in_=idb_d[8:9, :]).then_inc(dma_sem, 16)
                sync.dma_start(out=idr_sb[:, :], in_=idr_d[:, :]).then_inc(dma_sem, 16)
                sync.dma_start(out=bias_sb[:, :], in_=wq[1536:1537, :]).then_inc(dma_sem, 16)

                sync.wait_ge(h_sem, 2 * t_steps)
                sync.dma_start(out=out_d[:, :], in_=hlastT[:, :]).then_inc(dma_sem, 16)

            @block.tensor
            def _(tensor):
                tensor.wait_ge(dma_sem, 16 * N_DMAS)

                def xbias_mms(t, banks=(0, 1)):
                    # bias openers + x part of z(t); runs two steps ahead
                    zA, zB = zb[(t % 2) * 2], zb[(t % 2) * 2 + 1]
                    for bk, z in ((0, zA), (1, zB)):
                        if bk not in banks:
                            continue
                        # M=128 opener: bias into rows 0:8, ZEROS elsewhere
                        # (clears the bank so start=False matmuls accumulate)
                        tensor.matmul(
                            z[:, :],
                            ones_sb[0:1, :],
                            bias_sb[0:1, (4 * bk) * NT:(4 * bk) * NT + NT],
                            start=True, stop=False,
                            skip_group_check=True,
                        )
                        for cg in range(1, 4):
                            ncol = (4 * bk + cg) * NT
                            tensor.matmul(
                                z[GOFF[cg]:GOFF[cg] + BL, :],
                                ones_sb[0:1, 0:8],
                                bias_sb[0:1, ncol:ncol + NT],
                                start=False, stop=False,
                                tile_position=(0, GOFF[cg]),
                                skip_group_check=True,
                            )
                        for kc in range(NKX):
                            lhsT = x_sb[:, kc * TB + t * BL: kc * TB + (t + 1) * BL]
                            for cg in range(4):
                                ncol = (4 * bk + cg) * NT
                                last = (t == 0 and kc == NKX - 1 and cg == 3)
                                ins = tensor.matmul(
                                    z[GOFF[cg]:GOFF[cg] + BL, :],
                                    lhsT,
                                    w_sb[:, kc * 4096 + ncol: kc * 4096 + ncol + NT],
                                    start=False, stop=last,
                                    tile_position=(0, GOFF[cg]),
                                    skip_group_check=True,
                                )
                                if last:
                                    ins.then_inc(mm_sem, 1)

                def h_mms(t):
                    zA, zB = zb[(t % 2) * 2], zb[(t % 2) * 2 + 1]
                    rd = (t + 1) % 2   # hT written at step t-1
                    for half in range(2):
                        tensor.wait_ge(h_sem, 2 * (t - 1) + half + 1)
                        for j in range(half * 4, half * 4 + 4):
                            for bk, z in ((0, zA), (1, zB)):
                                for cg in range(4):
                                    ncol = (4 * bk + cg) * NT
                                    last = (j == 7 and cg == 3)
                                    ins = tensor.matmul(
                                        z[GOFF[cg]:GOFF[cg] + BL, :],
                                        hT[rd][:, j * BL:(j + 1) * BL],
                                        w_sb[:, (NKX + j) * 4096 + ncol:
                                             (NKX + j) * 4096 + ncol + NT],
                                        start=False, stop=last,
                                        tile_position=(0, GOFF[cg]),
                                        skip_group_check=True,
                                    )
                                    if last:
                                        ins.then_inc(mm_sem, 1)

                def transposes_bank(t, bk):
                    # per-bank sT cols: i 0:32 | f 32:64 | o 64:96 | g 96:128.
                    # In this kernel's instruction mix any transpose pinned to
                    # a PE row-group other than 0 aborts on HW, so f/g/o gate
                    # rows (partitions 32/64/96) are DVE-moved to partition 0
                    # (fgo_sb) and every transpose runs in row-group 0.
                    # The caller interleaves this bank's next-next-step x
                    # matmuls between the i and f/g/o transposes: they are
                    # legal as soon as this bank's sigmoid has read z, they
                    # give the DVE moves time to land, and they keep the HAM
                    # clock-gate fed through the transpose window.
                    for u4 in range(4):
                        tensor.matmul(
                            sT[t % 2][bk][:, u4 * 8:(u4 + 1) * 8],
                            s_sb[t % 2][bk][0:BL, u4 * 128:(u4 + 1) * 128],
                            idr_sb[0:BL, :],
                            start=True, stop=True,
                            is_transpose=True,
                            tile_position=(0, 0),
                            skip_group_check=True,
                        )

                def transposes_fgo(t, bk):
                    for gi, dst in ((0, 32), (1, 96), (2, 64)):   # f, g, o
                        tensor.wait_ge(oc_sem, 6 * t + 3 * bk + gi + 1)
                        for u4 in range(4):
                            ins = tensor.matmul(
                                sT[t % 2][bk][:, dst + u4 * 8: dst + (u4 + 1) * 8],
                                fgo_sb[bk][0:BL, gi * NT + u4 * 128:
                                           gi * NT + (u4 + 1) * 128],
                                idr_sb[0:BL, :],
                                start=True, stop=True,
                                is_transpose=True,
                                tile_position=(0, 0),
                                skip_group_check=True,
                            )
                    ins.then_inc(tr_sem, 1)

                xbias_mms(0)
                xbias_mms(1)
                for t in range(t_steps):
                    if t > 0:
                        h_mms(t)
                    for bk in range(2):
                        tensor.wait_ge(sig_sem, 2 * t + bk + 1)
                        transposes_bank(t, bk)
                        if t + 2 < t_steps:
                            xbias_mms(t + 2, banks=(bk,))
                        transposes_fgo(t, bk)

            @block.scalar
            def _(scalar):
                for t in range(t_steps):
                    for bk in range(2):
                        scalar.wait_ge(mm_sem, 2 * t + bk + 1)
                        if t >= 2:
                            scalar.wait_ge(tr_sem, 2 * (t - 2) + bk + 1)
                            scalar.wait_ge(oc_sem, 6 * (t - 2) + 3 * bk + 3)
                        nc.scalar.activation(
                            s_sb[t % 2][bk][:, :], zb[(t % 2) * 2 + bk][:, :],
                            AF.Sigmoid,
                        ).then_inc(sig_sem, 1)
                    for bk in range(2):
                        scalar.wait_ge(c_sem, 2 * t + bk + 1)
                        if t >= 1:
                            scalar.wait_ge(h_sem, 2 * (t - 1) + bk + 1)
                        nc.scalar.activation(
                            tc_sb[:, bk * 32:(bk + 1) * 32], cT[:, bk * 32:(bk + 1) * 32],
                            AF.Tanh,
                        ).then_inc(tanh_sem, 1)

            @block.vector
            def _(vector):
                ALU = mybir.AluOpType
                nc.vector.memset(cT[:, :], 0.0)
                vector.drain()
                for t in range(t_steps):
                    for bk in range(2):
                        # move f/g/o gate rows (partitions 32/64/96) down to
                        # partition 0; per-gate sem incs let the PE transposes
                        # pipeline behind each move
                        vector.wait_ge(sig_sem, 2 * t + bk + 1)
                        if t >= 1:
                            vector.wait_ge(tr_sem, 2 * (t - 1) + bk + 1)
                        for gi in range(3):
                            nc.vector.tensor_scalar(
                                fgo_sb[bk][:, gi * NT:(gi + 1) * NT],
                                s_sb[t % 2][bk][32 * (gi + 1):32 * (gi + 2), :],
                                1.0, 0.0, ALU.mult, ALU.add,
                            ).then_inc(oc_sem, 1)
                    for bk in range(2):
                        sTt = sT[t % 2][bk]
                        lo, hi = bk * 32, (bk + 1) * 32
                        vector.wait_ge(tr_sem, 2 * t + bk + 1)
                        # gm = tanh(zg) = 2*sigmoid(2 zg) - 1
                        nc.vector.tensor_scalar(
                            gm_sb[:, lo:hi], sTt[:, 96:128],
                            2.0, -1.0, ALU.mult, ALU.add,
                        )
                        nc.vector.tensor_mul(
                            c1_sb[:, lo:hi], sTt[:, 32:64], cT[:, lo:hi])
                        vector.drain()
                        nc.vector.tensor_mul(
                            t1_sb[:, lo:hi], gm_sb[:, lo:hi], sTt[:, 0:32])
                        vector.drain()
                        nc.vector.tensor_add(
                            cT[:, lo:hi], c1_sb[:, lo:hi], t1_sb[:, lo:hi],
                        ).then_inc(c_sem, 1)
                        vector.drain()
                    for bk in range(2):
                        sTt = sT[t % 2][bk]
                        lo, hi = bk * 32, (bk + 1) * 32
                        vector.wait_ge(tanh_sem, 2 * t + bk + 1)
                        dst = hT[t % 2] if t < t_steps - 1 else hlastT
                        nc.vector.tensor_mul(
                            dst[:, lo:hi], sTt[:, 64:96], tc_sb[:, lo:hi],
                        ).then_inc(h_sem, 1)

    return nc


def _prep_w(Wx, Wh, b):
    """[Wx; Wh; b] rows, columns permuted to per-bank [i|f|g|o] blocks,
    g-gate columns pre-scaled by 2 (tanh(x) = 2*sigmoid(2x) - 1)."""
    bf16 = ml_dtypes.bfloat16
    Wfull = np.concatenate([Wx, Wh, b[None, :]], axis=0).astype(np.float32)
    cols = []
    for bank in range(2):
        u0, u1 = bank * NT, (bank + 1) * NT
        cols.append(np.arange(0 * U + u0, 0 * U + u1))       # i
        cols.append(np.arange(1 * U + u0, 1 * U + u1))       # f
        cols.append(np.arange(2 * U + u0, 2 * U + u1))       # g
        cols.append(np.arange(3 * U + u0, 3 * U + u1))       # o
    perm = np.concatenate(cols)
    Wp = Wfull[:, perm].copy()
    for bank in range(2):
        g0 = bank * 4 * NT + 2 * NT
        Wp[:, g0:g0 + NT] *= 2.0
    return np.ascontiguousarray(Wp).astype(bf16)


def _make_in_maps(x, Wx, Wh, b):
    bf16 = ml_dtypes.bfloat16
    t_steps = x.shape[1]
    Wp = _prep_w(Wx, Wh, b)
    idb = np.zeros((9, 128), dtype=bf16)
    for i in range(8):
        idb[i, i] = 1.0
        idb[8, i] = 1.0
    idr = np.zeros((104, 8), dtype=bf16)
    for off in GOFF:
        for i in range(8):
            idr[off + i, i] = 1.0
    in_maps = []
    for core in range(NCORES):
        xs = x[core * BL:(core + 1) * BL].astype(np.float32)      # [BL, T, D]
        xt = np.ascontiguousarray(np.transpose(xs, (2, 1, 0)))    # [D, T, BL]
        xt = xt.reshape(NKX, 128, t_steps, BL)
        in_maps.append({
            "wq": Wp,
            "xq": np.ascontiguousarray(xt).astype(bf16),
            "idb": idb,
            "idr": idr,
        })
    return in_maps


def _decode_out(o):
    """[128, 64] unit-major -> [BL, U] batch-major: h[b, u] = o[u%128, (u//128)*8+b]."""
    o = np.asarray(o, dtype=np.float32).reshape(128, 8, 8)    # [p, chunk, b]
    return np.ascontiguousarray(np.transpose(o, (2, 1, 0))).reshape(8, U)


def kernel(x, Wx, Wh, b):
    x = np.asarray(x, dtype=np.float32)
    Wx = np.asarray(Wx, dtype=np.float32)
    Wh = np.asarray(Wh, dtype=np.float32)
    b = np.asarray(b, dtype=np.float32)
    t_steps = x.shape[1]

    in_maps = _make_in_maps(x, Wx, Wh, b)
    nc = _build_nc(t_steps)

    from concourse.bass_utils import run_bass_kernel_spmd
    core_ids = list(range(NCORES))
    res = run_bass_kernel_spmd(nc, in_maps, core_ids,
                               trace=bool(globals().get("TRACE", False)))
    globals()["LAST_EXEC_NS"] = res.exec_time_ns

    h_parts = [_decode_out(res.results[i]["out"]) for i in core_ids]
    return np.concatenate(h_parts, axis=0)


# revision 21
# speedup vs baseline: 1.0232x; 1.0232x over previous
"""Trainium2 Bass kernel for nn_BasicLSTM (B=64, T=512, D=512, U=1024).

Data-parallel over batch across 8 NeuronCores (8 sequences per core, the
recurrence fully local per core — no cross-core communication).

Per-core schedule, per step t:
  z(t) = [x_t, h, 1] @ W accumulates in two PSUM banks (units 0:512 and
  512:1024), each holding the four gates (i, f, g, o) at partition
  offsets (0, 32, 64, 96) via 4-way PE column-group packing (M=8).
  The x/bias part runs two steps ahead (it fills the PE during the
  cell-update tail and keeps the HAM clock-gate from re-throttling);
  the h part uses hT produced by the previous step's tail.

  The cell update runs in unit-major layout: sigmoid(z) (one ACT pass
  per bank; g-gate columns host-prescaled by 2 so tanh(x) =
  2*sigmoid(2x)-1 needs no second ACT pass) is PE-transposed into
  sT[128 units, gate*32 + chunk*8 + batch] so every DVE tail op is a
  [128, 32] tensor (all 128 lanes busy) instead of [8, 512] (8 lanes).
  The tail's hT output IS the lhsT layout the next h-matmul needs.

  HW quirk: in this instruction mix, PE transposes pinned to row-groups
  other than 0 abort the NEFF, so the f/g/o gate rows (partitions
  32/64/96) are DVE-moved down to partition 0 (fgo_sb) first; the moves
  are pipelined with the transposes at per-gate granularity.

Matmul operands are bf16 (fp32 PSUM accumulation); c stays fp32.
"""

import numpy as np
import ml_dtypes

B, T, D, U = 64, 512, 512, 1024
NCORES = 8
BL = B // NCORES          # 8 sequences per core
NKX = D // 128            # 4 x K-chunks
NKH = U // 128            # 8 h K-chunks
NT = 512                  # N-tile width (one PSUM bank)
GOFF = (0, 32, 64, 96)    # PSUM partition offset per col-group (i,f,g,o)


def _build_nc(t_steps=T):
    import concourse.bass as bass
    import concourse.mybir as mybir

    f32, bf16 = mybir.dt.float32, mybir.dt.bfloat16
    AF = mybir.ActivationFunctionType
    ALU = mybir.AluOpType

    TB = t_steps * BL

    nc = bass.Bass(num_devices=NCORES)
    wq = nc.declare_dram_parameter("wq", [1537, 4096], bf16, isOutput=False)
    xq = nc.declare_dram_parameter("xq", [NKX, 128, t_steps, BL], bf16, isOutput=False)
    idb_d = nc.declare_dram_parameter("idb", [9, 128], bf16, isOutput=False)
    idr_d = nc.declare_dram_parameter("idr", [104, 8], bf16, isOutput=False)
    out_d = nc.declare_dram_parameter("out", [128, 64], f32, isOutput=True)

    from contextlib import ExitStack
    ctx = ExitStack()
    sb = lambda shape, dt, name: ctx.enter_context(nc.sbuf_tensor(name, shape, dt))
    ps = lambda shape, dt, name: ctx.enter_context(nc.psum_tensor(name, shape, dt))
    sem = lambda name: ctx.enter_context(nc.semaphore(name))

    with ctx:
        w_sb = sb([128, (NKX + NKH) * 4096], bf16, "w_sb")   # Wx then Wh chunks
        bias_sb = sb([1, 4096], bf16, "bias_sb")
        x_sb = sb([128, NKX * TB], bf16, "x_sb")
        s_sb = [[sb([128, NT], bf16, f"s{d}{k}") for k in range(2)] for d in range(2)]
        ones_sb = sb([1, 128], bf16, "ones_sb")
        idr_sb = sb([104, 8], bf16, "idr_sb")
        fgo_sb = [sb([32, 3 * NT], bf16, f"fgo{i}") for i in range(2)]
        gm_sb = sb([128, 64], bf16, "gm_sb")
        t1_sb = sb([128, 64], bf16, "t1_sb")
        c1_sb = sb([128, 64], f32, "c1_sb")
        cT = sb([128, 64], f32, "cT")
        tc_sb = sb([128, 64], bf16, "tc_sb")
        hT = [sb([128, 64], bf16, f"hT{i}") for i in range(2)]
        hlastT = sb([128, 64], f32, "hlastT")

        zb = [ps([128, NT], f32, f"zb{i}") for i in range(4)]
        # full-bank sized so no two share a physical PSUM bank (PE writes one
        # while DVE reads another; same-bank overlap is fatal on HW)
        sT = [[ps([128, 1024], bf16, f"sT{d}{k}") for k in range(2)] for d in range(2)]

        dma_sem = sem("dma_sem")
        mm_sem = sem("mm_sem")
        sig_sem = sem("sig_sem")
        oc_sem = sem("oc_sem")     # 6 incs/step: (fA,gA,oA,fB,gB,oB)
        tr_sem = sem("tr_sem")
        c_sem = sem("c_sem")
        tanh_sem = sem("tanh_sem")
        h_sem = sem("h_sem")

        N_DMAS = (NKX + NKH) + NKX + 3   # w, x, idb, idr, bias

        with nc.Block() as block:

            @block.sync
            def _(sync):
                for kc in range(NKX + NKH):
                    sync.dma_start(
                        out=w_sb[:, kc * 4096:(kc + 1) * 4096],
                        in_=wq[kc * 128:(kc + 1) * 128, :],
                    ).then_inc(dma_sem, 16)
                for kc in range(NKX):
                    sync.dma_start(
                        out=x_sb[:, kc * TB:(kc + 1) * TB],
                        in_=xq[kc],
                    ).then_inc(dma_sem, 16)
                sync.dma_start(out=ones_sb[:, :], in_=idb_d[8:9, :]).then_inc(dma_sem, 16)
                sync.dma_start(out=idr_sb[:, :], in_=idr_d[:, :]).then_inc(dma_sem, 16)
                sync.dma_start(out=bias_sb[:, :], in_=wq[1536:1537, :]).then_inc(dma_sem, 16)

                sync.wait_ge(h_sem, 2 * t_steps)
                sync.dma_start(out=out_d[:, :], in_=hlastT[:, :]).then_inc(dma_sem, 16)

            @block.tensor
            def _(tensor):
                tensor.wait_ge(dma_sem, 16 * N_DMAS)

                def xbias_mms(t, banks=(0, 1)):
                    # bias openers + x part of z(t); runs two steps ahead
                    zA, zB = zb[(t % 2) * 2], zb[(t % 2) * 2 + 1]
                    for bk, z in ((0, zA), (1, zB)):
                        if bk not in banks:
                            continue
                        # M=128 opener: bias into rows 0:8, ZEROS elsewhere
                        # (clears the bank so start=False matmuls accumulate)
                        tensor.matmul(
                            z[:, :],
                            ones_sb[0:1, :],
                            bias_sb[0:1, (4 * bk) * NT:(4 * bk) * NT + NT],
                            start=True, stop=False,
                            skip_group_check=True,
                        )
                        for cg in range(1, 4):
                            ncol = (4 * bk + cg) * NT
                            tensor.matmul(
                                z[GOFF[cg]:GOFF[cg] + BL, :],
                                ones_sb[0:1, 0:8],
                                bias_sb[0:1, ncol:ncol + NT],
                                start=False, stop=False,
                                tile_position=(0, GOFF[cg]),
                                skip_group_check=True,
                            )
                        for kc in range(NKX):
                            lhsT = x_sb[:, kc * TB + t * BL: kc * TB + (t + 1) * BL]
                            for cg in range(4):
                                ncol = (4 * bk + cg) * NT
                                last = (t == 0 and kc == NKX - 1 and cg == 3)
                                ins = tensor.matmul(
                                    z[GOFF[cg]:GOFF[cg] + BL, :],
                                    lhsT,
                                    w_sb[:, kc * 4096 + ncol: kc * 4096 + ncol + NT],
                                    start=False, stop=last,
                                    tile_position=(0, GOFF[cg]),
                                    skip_group_check=True,
                                )
                                if last:
                                    ins.then_inc(mm_sem, 1)

                def h_mms(t):
                    zA, zB = zb[(t % 2) * 2], zb[(t % 2) * 2 + 1]
                    rd = (t + 1) % 2   # hT written at step t-1
                    # bank-major within each half: alternating PSUM banks per
                    # matmul group trips the HAM psum-queue-cycling throttle
                    for half in range(2):
                        tensor.wait_ge(h_sem, 2 * (t - 1) + half + 1)
                        for bk, z in ((0, zA), (1, zB)):
                            for j in range(half * 4, half * 4 + 4):
                                for cg in range(4):
                                    ncol = (4 * bk + cg) * NT
                                    last = (half == 1 and j == 7 and cg == 3)
                                    ins = tensor.matmul(
                                        z[GOFF[cg]:GOFF[cg] + BL, :],
                                        hT[rd][:, j * BL:(j + 1) * BL],
                                        w_sb[:, (NKX + j) * 4096 + ncol:
                                             (NKX + j) * 4096 + ncol + NT],
                                        start=False, stop=last,
                                        tile_position=(0, GOFF[cg]),
                                        skip_group_check=True,
                                    )
                                    if last:
                                        ins.then_inc(mm_sem, 1)

                def transposes_bank(t, bk):
                    # per-bank sT cols: i 0:32 | f 32:64 | o 64:96 | g 96:128.
                    # In this kernel's instruction mix any transpose pinned to
                    # a PE row-group other than 0 aborts on HW, so f/g/o gate
                    # rows (partitions 32/64/96) are DVE-moved to partition 0
                    # (fgo_sb) and every transpose runs in row-group 0.
                    # The caller interleaves this bank's next-next-step x
                    # matmuls between the i and f/g/o transposes: they are
                    # legal as soon as this bank's sigmoid has read z, they
                    # give the DVE moves time to land, and they keep the HAM
                    # clock-gate fed through the transpose window.
                    for u4 in range(4):
                        tensor.matmul(
                            sT[t % 2][bk][:, u4 * 8:(u4 + 1) * 8],
                            s_sb[t % 2][bk][0:BL, u4 * 128:(u4 + 1) * 128],
                            idr_sb[0:BL, :],
                            start=True, stop=True,
                            is_transpose=True,
                            tile_position=(0, 0),
                            skip_group_check=True,
                        )

                def transposes_fgo(t, bk):
                    for gi, dst in ((0, 32), (1, 96), (2, 64)):   # f, g, o
                        tensor.wait_ge(oc_sem, 6 * t + 3 * bk + gi + 1)
                        for u4 in range(4):
                            ins = tensor.matmul(
                                sT[t % 2][bk][:, dst + u4 * 8: dst + (u4 + 1) * 8],
                                fgo_sb[bk][0:BL, gi * NT + u4 * 128:
                                           gi * NT + (u4 + 1) * 128],
                                idr_sb[0:BL, :],
                                start=True, stop=True,
                                is_transpose=True,
                                tile_position=(0, 0),
                                skip_group_check=True,
                            )
                    ins.then_inc(tr_sem, 1)

                xbias_mms(0)
                xbias_mms(1)
                for t in range(t_steps):
                    if t > 0:
                        h_mms(t)
                    for bk in range(2):
                        tensor.wait_ge(sig_sem, 2 * t + bk + 1)
                        transposes_bank(t, bk)
                        if t + 2 < t_steps:
                            xbias_mms(t + 2, banks=(bk,))
                        transposes_fgo(t, bk)

            @block.scalar
            def _(scalar):
                for t in range(t_steps):
                    for bk in range(2):
                        scalar.wait_ge(mm_sem, 2 * t + bk + 1)
                        if t >= 2:
                            scalar.wait_ge(tr_sem, 2 * (t - 2) + bk + 1)
                            scalar.wait_ge(oc_sem, 6 * (t - 2) + 3 * bk + 3)
                        nc.scalar.activation(
                            s_sb[t % 2][bk][:, :], zb[(t % 2) * 2 + bk][:, :],
                            AF.Sigmoid,
                        ).then_inc(sig_sem, 1)
                    for bk in range(2):
                        scalar.wait_ge(c_sem, 2 * t + bk + 1)
                        if t >= 1:
                            scalar.wait_ge(h_sem, 2 * (t - 1) + bk + 1)
                        nc.scalar.activation(
                            tc_sb[:, bk * 32:(bk + 1) * 32], cT[:, bk * 32:(bk + 1) * 32],
                            AF.Tanh,
                        ).then_inc(tanh_sem, 1)

            @block.vector
            def _(vector):
                ALU = mybir.AluOpType
                nc.vector.memset(cT[:, :], 0.0)
                vector.drain()
                for t in range(t_steps):
                    for bk in range(2):
                        # move f/g/o gate rows (partitions 32/64/96) down to
                        # partition 0; per-gate sem incs let the PE transposes
                        # pipeline behind each move
                        vector.wait_ge(sig_sem, 2 * t + bk + 1)
                        if t >= 1:
                            vector.wait_ge(tr_sem, 2 * (t - 1) + bk + 1)
                        for gi in range(3):
                            nc.vector.tensor_scalar(
                                fgo_sb[bk][:, gi * NT:(gi + 1) * NT],
                                s_sb[t % 2][bk][32 * (gi + 1):32 * (gi + 2), :],
                                1.0, 0.0, ALU.mult, ALU.add,
                            ).then_inc(oc_sem, 1)
                    for bk in range(2):
                        sTt = sT[t % 2][bk]
                        lo, hi = bk * 32, (bk + 1) * 32
                        vector.wait_ge(tr_sem, 2 * t + bk + 1)
                        # gm = tanh(zg) = 2*sigmoid(2 zg) - 1
                        nc.vector.tensor_scalar(
                            gm_sb[:, lo:hi], sTt[:, 96:128],
                            2.0, -1.0, ALU.mult, ALU.add,
                        )
                        nc.vector.tensor_mul(
                            c1_sb[:, lo:hi], sTt[:, 32:64], cT[:, lo:hi])
                        vector.drain()
                        nc.vector.tensor_mul(
                            t1_sb[:, lo:hi], gm_sb[:, lo:hi], sTt[:, 0:32])
                        vector.drain()
                        nc.vector.tensor_add(
                            cT[:, lo:hi], c1_sb[:, lo:hi], t1_sb[:, lo:hi],
                        ).then_inc(c_sem, 1)
                        vector.drain()
                    for bk in range(2):
                        sTt = sT[t % 2][bk]
                        lo, hi = bk * 32, (bk + 1) * 32
                        vector.wait_ge(tanh_sem, 2 * t + bk + 1)
                        dst = hT[t % 2] if t < t_steps - 1 else hlastT
                        nc.vector.tensor_mul(
                            dst[:, lo:hi], sTt[:, 64:96], tc_sb[:, lo:hi],
                        ).then_inc(h_sem, 1)

    return nc


def _prep_w(Wx, Wh, b):
    """[Wx; Wh; b] rows, columns permuted to per-bank [i|f|g|o] blocks,
    g-gate columns pre-scaled by 2 (tanh(x) = 2*sigmoid(2x) - 1)."""
    bf16 = ml_dtypes.bfloat16
    Wfull = np.concatenate([Wx, Wh, b[None, :]], axis=0).astype(np.float32)
    cols = []
    for bank in range(2):
        u0, u1 = bank * NT, (bank + 1) * NT
        cols.append(np.arange(0 * U + u0, 0 * U + u1))       # i
        cols.append(np.arange(1 * U + u0, 1 * U + u1))       # f
        cols.append(np.arange(2 * U + u0, 2 * U + u1))       # g
        cols.append(np.arange(3 * U + u0, 3 * U + u1))       # o
    perm = np.concatenate(cols)
    Wp = Wfull[:, perm].copy()
    for bank in range(2):
        g0 = bank * 4 * NT + 2 * NT
        Wp[:, g0:g0 + NT] *= 2.0
    return np.ascontiguousarray(Wp).astype(bf16)


def _make_in_maps(x, Wx, Wh, b):
    bf16 = ml_dtypes.bfloat16
    t_steps = x.shape[1]
    Wp = _prep_w(Wx, Wh, b)
    idb = np.zeros((9, 128), dtype=bf16)
    for i in range(8):
        idb[i, i] = 1.0
        idb[8, i] = 1.0
    idr = np.zeros((104, 8), dtype=bf16)
    for off in GOFF:
        for i in range(8):
            idr[off + i, i] = 1.0
    in_maps = []
    for core in range(NCORES):
        xs = x[core * BL:(core + 1) * BL].astype(np.float32)      # [BL, T, D]
        xt = np.ascontiguousarray(np.transpose(xs, (2, 1, 0)))    # [D, T, BL]
        xt = xt.reshape(NKX, 128, t_steps, BL)
        in_maps.append({
            "wq": Wp,
            "xq": np.ascontiguousarray(xt).astype(bf16),
            "idb": idb,
            "idr": idr,
        })
    return in_maps


def _decode_out(o):
    """[128, 64] unit-major -> [BL, U] batch-major: h[b, u] = o[u%128, (u//128)*8+b]."""
    o = np.asarray(o, dtype=np.float32).reshape(128, 8, 8)    # [p, chunk, b]
    return np.ascontiguousarray(np.transpose(o, (2, 1, 0))).reshape(8, U)


def kernel(x, Wx, Wh, b):
    x = np.asarray(x, dtype=np.float32)
    Wx = np.asarray(Wx, dtype=np.float32)
    Wh = np.asarray(Wh, dtype=np.float32)
    b = np.asarray(b, dtype=np.float32)
    t_steps = x.shape[1]

    in_maps = _make_in_maps(x, Wx, Wh, b)
    nc = _build_nc(t_steps)

    from concourse.bass_utils import run_bass_kernel_spmd
    core_ids = list(range(NCORES))
    res = run_bass_kernel_spmd(nc, in_maps, core_ids,
                               trace=bool(globals().get("TRACE", False)))
    globals()["LAST_EXEC_NS"] = res.exec_time_ns

    h_parts = [_decode_out(res.results[i]["out"]) for i in core_ids]
    return np.concatenate(h_parts, axis=0)
